# revision 1
# baseline (speedup 1.0000x reference)
"""Trainium2 Bass kernel for nn_HLSTransformer (2-block self-attention encoder).

Contract: kernel(**inputs) takes FULL inputs (see shapes below), returns FULL
output [32, 1] float32.  Data-parallel over batch: 4 samples per core x 8 cores.

All on-device activations live in "T layout": [H=64 partitions, N=1024 free],
two samples packed per 128-partition tile (rows 0-63 = even sample, 64-127 =
odd sample of the pair).

Math notes:
  - softmax((s - max + mask) * SCALE) == softmax((s + mask) * SCALE) exactly
    (row-constant shift cancels), and the arguments here are small enough
    (|G*SCALE| < ~4) that fp32 exp cannot overflow, so no max pass is needed.
  - LayerNorm is over the whole [N, H] slab per sample -> scalar mean/var.
  - rsqrt(var+eps) = exp(-0.5 * ln(var+eps)) keeps the ACT engine in the
    natural_log_exp_and_others table set (one table load for the kernel).
"""

import sys

import numpy as np
import ml_dtypes

if "/opt/trn_rl_repo" not in sys.path:
    sys.path.insert(0, "/opt/trn_rl_repo")

import concourse.bass as bass
import concourse.bacc as bacc
import concourse.tile as tile
from concourse import mybir
from concourse.bass_utils import run_bass_kernel_spmd

F32 = mybir.dt.float32
BF16 = mybir.dt.bfloat16
AF = mybir.ActivationFunctionType

# Force Exp and Ln to resolve to the one table set containing both, so the
# ACT engine never thrashes ACT_TABLE_LOADs between them.
_orig_gat = bacc.get_activation_tables
def _gat_patched(arch):
    out = {}
    for name, fns in _orig_gat(arch).items():
        fns = set(fns)
        if name != "natural_log_exp_and_others":
            fns.discard(mybir.ActivationFunctionType.Exp)
            fns.discard(mybir.ActivationFunctionType.Ln)
        out[name] = fns
    return out
bacc.get_activation_tables = _gat_patched
ALU = mybir.AluOpType

B, N, F_IN, H = 32, 1024, 256, 64
NCORES = 8
S = B // NCORES            # samples per core
NPAIR = S // 2             # sample pairs per core
EPS = 1e-5
SCALE = float(1.0 / np.sqrt(np.float32(N)))
NH = 512                   # free-dim half (PSUM bank)
NC = 8                     # 128-chunks along N


def _ln_scalar_chain(nc, small, pmisc, selsum, selbc, eps2, m2):
    """From per-partition [128,2] (mean_p, ex2_p) in sbuf `m2`, produce the
    broadcast [128, 2] sbuf tile (negmu, rho) for the fused normalize apply.
    selsum entries are 1/64 so the partition-reduce matmul directly averages."""
    stps = pmisc.tile([2, 2], F32, tag="misc")
    nc.tensor.matmul(stps[:, :], selsum[:, :], m2[:, :])
    sv = small.tile([2, 2], F32, tag="sv")          # (mean, ex2) per sample
    nc.vector.tensor_copy(sv[:, :], stps[:, :])
    pb = small.tile([2, 2], F32, tag="pb")          # cols: [negmu, rho]
    msq = small.tile([2, 1], F32, tag="msq")
    nc.vector.tensor_tensor(msq[:, :], sv[:, 0:1], sv[:, 0:1], op=ALU.mult)
    var = small.tile([2, 1], F32, tag="var")
    nc.vector.tensor_tensor(var[:, :], sv[:, 1:2], msq[:, :], op=ALU.subtract)
    lnv = small.tile([2, 1], F32, tag="lnv")
    nc.scalar.activation(lnv[:, :], var[:, :], AF.Ln, bias=eps2[:, :])
    nc.scalar.activation(pb[:, 1:2], lnv[:, :], AF.Exp, scale=-0.5)
    nc.vector.tensor_scalar(pb[:, 0:1], sv[:, 0:1], -1.0, None, op0=ALU.mult)
    bcps = pmisc.tile([128, 2], F32, tag="misc")
    nc.tensor.matmul(bcps[:, :], selbc[:, :], pb[:, :])
    bc = small.tile([128, 2], F32, tag="bc")
    nc.vector.tensor_copy(bc[:, :], bcps[:, :])
    return bc


def _layernorm(nc, small, pmisc, selsum, selbc, eps2, v_pair, out_pair, gb):
    """out = (v - mu) * rho per sample (pair tile [128, 1024]); optional
    gamma/beta affine (gb = (gammaT2, betaT2) sbuf tiles or None)."""
    st6 = small.tile([128, 12], F32, tag="st6")
    nc.vector.bn_stats(st6[:, 0:6], v_pair[:, 0:NH])
    nc.vector.bn_stats(st6[:, 6:12], v_pair[:, NH:N])
    ag = small.tile([128, 2], F32, tag="ag")
    nc.vector.bn_aggr(ag[:, :], st6[:, :])
    m2 = small.tile([128, 2], F32, tag="m2")        # [mean_p, ex2_p]
    nc.vector.tensor_copy(m2[:, 0:1], ag[:, 0:1])
    msqp = small.tile([128, 1], F32, tag="msqp")
    nc.vector.tensor_tensor(msqp[:, :], ag[:, 0:1], ag[:, 0:1], op=ALU.mult)
    nc.vector.tensor_tensor(m2[:, 1:2], ag[:, 1:2], msqp[:, :], op=ALU.add)
    bc = _ln_scalar_chain(nc, small, pmisc, selsum, selbc, eps2, m2)
    nc.vector.tensor_scalar(
        out_pair[:, :], v_pair[:, :], bc[:, 0:1], bc[:, 1:2],
        op0=ALU.add, op1=ALU.mult,
    )
    if gb is not None:
        gam, bet = gb
        nc.vector.tensor_tensor(out_pair[:, :], out_pair[:, :], gam[:, :], op=ALU.mult)
        nc.vector.tensor_tensor(out_pair[:, :], out_pair[:, :], bet[:, :], op=ALU.add)


def build_nc(use_mask: bool, use_gb: bool) -> bass.Bass:
    nc = bacc.Bacc("TRN2", target_bir_lowering=False, debug=False, num_devices=NCORES)

    xT = nc.declare_dram_parameter("xT", [S, 2, 128, N], BF16, isOutput=False)
    we = nc.declare_dram_parameter("We", [2, 128, H], BF16, isOutput=False)
    w0 = nc.declare_dram_parameter("W0s", [128, H], BF16, isOutput=False)
    w1 = nc.declare_dram_parameter("W1s", [128, H], BF16, isOutput=False)
    wout = nc.declare_dram_parameter("Wouts", [128, 1], F32, isOutput=False)
    be2 = nc.declare_dram_parameter("be2", [128, 1], F32, isOutput=False)
    b02 = nc.declare_dram_parameter("b02", [128, 1], F32, isOutput=False)
    b12 = nc.declare_dram_parameter("b12", [128, 1], F32, isOutput=False)
    boutp = nc.declare_dram_parameter("bout", [1, 1], F32, isOutput=False)
    ident = nc.declare_dram_parameter("ident", [128, 128], BF16, isOutput=False)
    selsum_d = nc.declare_dram_parameter("selsum", [128, 2], F32, isOutput=False)
    selbc_d = nc.declare_dram_parameter("selbc", [2, 128], F32, isOutput=False)
    if use_gb:
        gT2_d = nc.declare_dram_parameter("gT2", [128, N], F32, isOutput=False)
        bT2_d = nc.declare_dram_parameter("bT2", [128, N], F32, isOutput=False)
    if use_mask:
        maskT_d = nc.declare_dram_parameter("maskT", [S, N, N], F32, isOutput=False)
    out_d = nc.declare_dram_parameter("out", [S, 1], F32, isOutput=True)

    with tile.TileContext(nc) as tc:
        with (
            tc.tile_pool(name="consts", bufs=1) as cp,
            tc.tile_pool(name="xt", bufs=4) as xp,
            tc.tile_pool(name="big", bufs=2) as bigp,
            tc.tile_pool(name="hn", bufs=4) as hnp,
            tc.tile_pool(name="e", bufs=3) as ep,
            tc.tile_pool(name="small", bufs=4) as small,
            tc.tile_pool(name="psg", bufs=2, space="PSUM") as pg,
            tc.tile_pool(name="psu", bufs=1, space="PSUM") as pu,
            tc.tile_pool(name="psmisc", bufs=2, space="PSUM") as pmisc,
        ):
            # ---- constants ----
            we_sb = cp.tile([128, 2, H], BF16, tag="we")
            nc.sync.dma_start(we_sb[:, :, :], we.rearrange("k p m -> p k m"))
            w0_sb = cp.tile([128, H], BF16, tag="w0")
            nc.sync.dma_start(w0_sb[:, :], w0[:, :])
            w1_sb = cp.tile([128, H], BF16, tag="w1")
            nc.sync.dma_start(w1_sb[:, :], w1[:, :])
            wo_sb = cp.tile([128, 1], F32, tag="wo")
            nc.sync.dma_start(wo_sb[:, :], wout[:, :])
            be_sb = cp.tile([128, 1], F32, tag="be")
            nc.sync.dma_start(be_sb[:, :], be2[:, :])
            b0_sb = cp.tile([128, 1], F32, tag="b0")
            nc.sync.dma_start(b0_sb[:, :], b02[:, :])
            b1_sb = cp.tile([128, 1], F32, tag="b1")
            nc.sync.dma_start(b1_sb[:, :], b12[:, :])
            bo_sb = cp.tile([1, 1], F32, tag="bo")
            nc.sync.dma_start(bo_sb[:, :], boutp[:, :])
            id_sb = cp.tile([128, 128], BF16, tag="id")
            nc.sync.dma_start(id_sb[:, :], ident[:, :])
            selsum = cp.tile([128, 2], F32, tag="ss")
            nc.sync.dma_start(selsum[:, :], selsum_d[:, :])
            selbc = cp.tile([2, 128], F32, tag="sb")
            nc.sync.dma_start(selbc[:, :], selbc_d[:, :])
            eps2 = cp.tile([2, 1], F32, tag="eps")
            nc.vector.memset(eps2[:, :], EPS)
            onesb = cp.tile([128, 64], F32, tag="onesb")
            nc.vector.memset(onesb[:, :], 1.0)
            zb128 = cp.tile([128, 1], F32, tag="zb128")
            nc.vector.memset(zb128[:, :], 0.0)
            gb = None
            if use_gb:
                gam = cp.tile([128, N], F32, tag="gam")
                nc.sync.dma_start(gam[:, :], gT2_d[:, :])
                bet = cp.tile([128, N], F32, tag="bet")
                nc.sync.dma_start(bet[:, :], bT2_d[:, :])
                gb = (gam, bet)

            # ---- embed: x_embT = relu(We.T @ xT + be), col-tiled pairs ----
            xemb = []
            for p in range(NPAIR):
                emb_ps = pg.tile([128, N], F32, tag="gram")
                for si, s in enumerate((2 * p, 2 * p + 1)):
                    xa = xp.tile([128, N], BF16, tag="xt")
                    xb = xp.tile([128, N], BF16, tag="xt")
                    nc.sync.dma_start(xa[:, :], xT[s, 0, :, :])
                    nc.sync.dma_start(xb[:, :], xT[s, 1, :, :])
                    for nh in range(2):
                        for k, xk in enumerate((xa, xb)):
                            nc.tensor.matmul(
                                emb_ps[64 * si:64 * si + 64, NH * nh:NH * nh + NH],
                                we_sb[:, k, :],
                                xk[:, NH * nh:NH * nh + NH],
                                start=(k == 0), stop=(k == 1),
                                tile_position=(0, 64 * si),
                            )
                xe = bigp.tile([128, N], BF16, tag="xemb", bufs=2)
                nc.vector.tensor_scalar(
                    xe[:, :], emb_ps[:, :], be_sb[:, :], 0.0,
                    op0=ALU.add, op1=ALU.max,
                )
                xemb.append(xe)

            # ---- two transformer blocks ----
            h_cur = list(xemb)
            fc_out = [None] * NPAIR
            for blk in range(2):
                wf_sb = w0_sb if blk == 0 else w1_sb
                bf_sb = b0_sb if blk == 0 else b1_sb
                for p in range(NPAIR):
                    hT = h_cur[p]
                    # -- hN: normal-layout h chunks [128, c, 65] with ones col --
                    hn = []
                    for si in range(2):
                        t = hnp.tile([128, NC, 65], BF16, tag="hn")
                        nc.gpsimd.memset(t[:, :, 64:65], 1.0)
                        for cq in range(2):  # batches of 4 chunks
                            tp = pmisc.tile([128, 256], BF16, tag="misc")
                            for j in range(4):
                                c = 4 * cq + j
                                nc.tensor.transpose(
                                    tp[:, 64 * j:64 * j + 64],
                                    hT[64 * si:64 * si + 64, 128 * c:128 * c + 128],
                                    id_sb[64 * si:64 * si + 64, 64 * si:64 * si + 64],
                                )
                            nc.vector.tensor_copy(
                                t[:, 4 * cq:4 * cq + 4, 0:64],
                                tp.rearrange("p (c m) -> p c m", c=4),
                            )
                        hn.append(t)

                    # -- attention: dense gram+exp phase, then dense U phase --
                    v_pair = bigp.tile([128, N], BF16, tag="v")
                    for nh in range(2):
                        ns = slice(NH * nh, NH * nh + NH)
                        elist = []
                        for c in range(8):
                            g = pg.tile([128, N], F32, tag="gram")
                            for si in range(2):
                                nc.tensor.matmul(
                                    g[:, NH * si:NH * si + NH],
                                    hT[64 * si:64 * si + 64, 128 * c:128 * c + 128],
                                    hT[64 * si:64 * si + 64, ns],
                                    start=True, stop=True,
                                )
                            if use_mask:
                                for si, s in enumerate((2 * p, 2 * p + 1)):
                                    mt = ep.tile([128, NH], BF16, tag="mt", bufs=3)
                                    nc.sync.dma_start(
                                        mt[:, :],
                                        maskT_d[s, 128 * c:128 * c + 128, ns],
                                    )
                                    nc.vector.tensor_tensor(
                                        g[:, NH * si:NH * si + NH],
                                        g[:, NH * si:NH * si + NH],
                                        mt[:, :], op=ALU.add,
                                    )
                            e = ep.tile([128, N], BF16, tag="e", bufs=12)
                            nc.scalar.activation(e[:, :], g[:, :], AF.Exp, scale=SCALE)
                            elist.append(e)
                        uA = pu.tile([128, NH], F32, tag="uA")
                        uB = pu.tile([128, NH], F32, tag="uB")
                        for c in range(8):
                            e = elist[c]
                            nc.tensor.matmul(
                                uA[0:65, :], hn[0][:, c, 0:65], e[:, 0:NH],
                                start=(c == 0), stop=(c == 7),
                                skip_group_check=True,
                            )
                            nc.tensor.matmul(
                                uB[64:128, :], hn[1][:, c, 0:64], e[:, NH:N],
                                start=(c == 0), stop=(c == 7),
                                tile_position=(0, 64),
                            )
                            nc.tensor.matmul(
                                uA[96:97, :], hn[1][:, c, 64:65], e[:, NH:N],
                                start=(c == 0), stop=(c == 7),
                                tile_position=(0, 96),
                                skip_group_check=True,
                            )
                        # -- normalize columns: v = U / Z --
                        zl = small.tile([128, NH], F32, tag="zl", bufs=2)
                        rz = small.tile([128, NH], F32, tag="rz", bufs=2)
                        nc.scalar.activation(zl[64:65, :], uA[64:65, :], AF.Ln,
                                             bias=zb128[64:65, :])
                        nc.scalar.activation(zl[96:97, :], uA[96:97, :], AF.Ln,
                                             bias=zb128[96:97, :])
                        nc.scalar.activation(rz[64:65, :], zl[64:65, :], AF.Exp,
                                             scale=-1.0, bias=zb128[64:65, :])
                        nc.scalar.activation(rz[96:97, :], zl[96:97, :], AF.Exp,
                                             scale=-1.0, bias=zb128[96:97, :])
                        # replicate rz rows via PE (ones-row matmuls)
                        rzp = pmisc.tile([128, NH], F32, tag="misc")
                        nc.tensor.matmul(
                            rzp[0:64, :], onesb[64:65, :], rz[64:65, :],
                            start=True, stop=True, tile_position=(64, 0),
                            skip_group_check=True,
                        )
                        nc.tensor.matmul(
                            rzp[64:128, :], onesb[96:97, :], rz[96:97, :],
                            start=True, stop=True, tile_position=(96, 64),
                            skip_group_check=True,
                        )
                        rzb = small.tile([128, NH], F32, tag="rzb", bufs=2)
                        nc.vector.tensor_copy(rzb[:, :], rzp[:, :])
                        nc.vector.tensor_tensor(
                            v_pair[0:64, ns], uA[0:64, :], rzb[0:64, :], op=ALU.mult,
                        )
                        nc.vector.tensor_tensor(
                            v_pair[64:128, ns], uB[64:128, :], rzb[64:128, :],
                            op=ALU.mult,
                        )
                    # -- residual (x_emb both blocks) + LN -> qkv --
                    nc.gpsimd.tensor_tensor(
                        v_pair[:, :], v_pair[:, :], xemb[p][:, :], op=ALU.add,
                    )
                    qkv = bigp.tile([128, N], BF16, tag="qkv")
                    _layernorm(nc, small, pmisc, selsum, selbc, eps2, v_pair, qkv, gb)

                    # -- FFN: f = relu(qkv @ Wf + bf); fc = LN(qkv + f) --
                    f_pair = bigp.tile([128, N], BF16, tag="f")
                    for nh in range(2):
                        ns = slice(NH * nh, NH * nh + NH)
                        fps = pmisc.tile([128, NH], F32, tag="misc")
                        nc.tensor.matmul(
                            fps[0:64, :], wf_sb[0:64, :], qkv[0:64, ns],
                            start=True, stop=True,
                        )
                        nc.tensor.matmul(
                            fps[64:128, :], wf_sb[64:128, :], qkv[64:128, ns],
                            start=True, stop=True, tile_position=(64, 64),
                        )
                        nc.vector.tensor_scalar(
                            f_pair[:, ns], fps[:, :], bf_sb[:, :], 0.0,
                            op0=ALU.add, op1=ALU.max,
                        )
                    nc.gpsimd.tensor_tensor(
                        f_pair[:, :], f_pair[:, :], qkv[:, :], op=ALU.add,
                    )
                    fc = bigp.tile([128, N], BF16, tag="fc")
                    _layernorm(nc, small, pmisc, selsum, selbc, eps2, f_pair, fc, gb)
                    fc_out[p] = fc
                h_cur = list(fc_out)

            # ---- pool + head ----
            for p in range(NPAIR):
                pooled = small.tile([128, 1], F32, tag="pool", bufs=2)
                nc.vector.reduce_sum(
                    pooled[:, :], h_cur[p][:, :], axis=mybir.AxisListType.X,
                )
                sc = pmisc.tile([1, 2], F32, tag="misc")
                nc.tensor.matmul(
                    sc[0:1, 0:1], wo_sb[0:64, :], pooled[0:64, :],
                    start=True, stop=True,
                )
                nc.tensor.matmul(
                    sc[0:1, 1:2], wo_sb[64:128, :], pooled[64:128, :],
                    start=True, stop=True,
                )
                # leaky_relu(z + bout) = max(z + bout, 0.01 * (z + bout))
                zb = small.tile([1, 2], F32, tag="zb", bufs=2)
                nc.vector.tensor_scalar(
                    zb[:, :], sc[:, :], bo_sb[0:1, :], None, op0=ALU.add,
                )
                res = small.tile([1, 2], F32, tag="res", bufs=2)
                nc.vector.tensor_scalar(
                    res[:, :], zb[:, :], 0.01, None, op0=ALU.mult,
                )
                nc.vector.tensor_tensor(
                    res[:, :], res[:, :], zb[:, :], op=ALU.max,
                )
                for si in range(2):
                    nc.sync.dma_start(
                        out_d[2 * p + si:2 * p + si + 1, :], res[0:1, si:si + 1],
                    )

    nc.compile()
    return nc


_NC_CACHE: dict = {}


def kernel(x, mask, We, be, gamma, beta, W0, b0, W1, b1, Wout, bout):
    x = np.ascontiguousarray(np.asarray(x, dtype=np.float32))
    mask = np.asarray(mask, dtype=np.float32)
    use_mask = bool(np.any(mask))
    use_gb = bool(np.any(np.asarray(gamma) != 1.0) or np.any(np.asarray(beta)))

    key = (use_mask, use_gb)
    if key not in _NC_CACHE:
        _NC_CACHE[key] = build_nc(use_mask, use_gb)
    nc = _NC_CACHE[key]

    ident = np.eye(128, dtype=np.float32)
    selsum = np.zeros((128, 2), dtype=np.float32)
    selsum[0:64, 0] = 1.0 / 64.0
    selsum[64:128, 1] = 1.0 / 64.0
    selbc = np.zeros((2, 128), dtype=np.float32)
    selbc[0, 0:64] = 1.0
    selbc[1, 64:128] = 1.0

    def stack2(v):
        v = np.asarray(v, dtype=np.float32).reshape(-1)
        return np.concatenate([v, v]).reshape(128, 1)

    common = {
        "We": np.ascontiguousarray(np.asarray(We, dtype=np.float32)).reshape(2, 128, H).astype(ml_dtypes.bfloat16),
        "W0s": np.concatenate([W0, W0]).astype(ml_dtypes.bfloat16),
        "W1s": np.concatenate([W1, W1]).astype(ml_dtypes.bfloat16),
        "Wouts": np.concatenate([Wout, Wout]).astype(np.float32),
        "be2": stack2(be), "b02": stack2(b0), "b12": stack2(b1),
        "bout": np.asarray(bout, dtype=np.float32).reshape(1, 1),
        "ident": ident.astype(ml_dtypes.bfloat16), "selsum": selsum, "selbc": selbc,
    }
    if use_gb:
        gT = np.ascontiguousarray(np.asarray(gamma, dtype=np.float32).T)
        bT = np.ascontiguousarray(np.asarray(beta, dtype=np.float32).T)
        common["gT2"] = np.concatenate([gT, gT]).astype(np.float32)
        common["bT2"] = np.concatenate([bT, bT]).astype(np.float32)

    in_maps = []
    for k in range(NCORES):
        xs = x[S * k:S * k + S]                       # [S, N, F_IN]
        xTs = np.ascontiguousarray(
            xs.transpose(0, 2, 1)).reshape(S, 2, 128, N)
        m = dict(common)
        m["xT"] = xTs.astype(ml_dtypes.bfloat16)
        if use_mask:
            m["maskT"] = np.ascontiguousarray(
                mask[S * k:S * k + S].transpose(0, 2, 1))
        in_maps.append(m)

    res = run_bass_kernel_spmd(nc, in_maps, list(range(NCORES)))
    global LAST_RESULT
    LAST_RESULT = res
    out = np.concatenate([res.results[k]["out"] for k in range(NCORES)], axis=0)
    return out.astype(np.float32)


LAST_RESULT = None



# revision 3
# speedup vs baseline: 1.4453x; 1.4453x over previous
"""Trainium2 Bass kernel for nn_HLSTransformer (2-block self-attention encoder).

Contract: kernel(**inputs) takes FULL inputs, returns FULL output [32, 1] f32.
Data-parallel over batch: 4 samples per core x 8 cores.

On-device layout: activations in "T layout" [H=64 partitions, N=1024 free],
two samples packed per 128-partition tile (rows 0-63 = even sample, 64-127 =
odd sample of the pair).

Attention design (per pair of samples, per block):
  - For each n-chunk c (8 chunks of 128 tokens) and sample s, one PSUM gram
    tile g = S_s[n in chunk, m in 0:1024] via two row-tiled matmuls (the two
    samples' matmuls run concurrently on disjoint PE row strips).
  - One big ACT exp per tile writes e (bf16 SBUF) and, via accum_out, the
    per-row partial sums = softmax denominators Z (scores are symmetric, so
    row sums equal the column sums needed later).  No max-subtract needed:
    softmax((s-max+mask)*SCALE) == softmax((s+mask)*SCALE) and |s*SCALE| is
    small enough that fp32 exp cannot overflow.
  - U = E @ h accumulated over chunks into two PSUM banks (m-halves); the two
    samples' U matmuls are column-tiled onto disjoint PE col strips and run
    concurrently.  U issue is delayed a few chunks so the previous pair's
    normalize can release the U banks without stalling the PE queue.
  - 1/Z: DVE reciprocal -> PE transpose -> SBUF-to-SBUF DMA gather into row
    form [2, 1024] -> one matmul pair against a {0,1} selector broadcasts it
    across partitions -> DVE multiply normalizes U.
  - LayerNorm over the whole [N, H] slab per sample -> scalar mean/var;
    rsqrt(var+eps) = exp(-0.5*ln(var+eps)) keeps ACT on one table set.

Emission interleaves the two pairs: while pair B's exp stream keeps the ACT
engine saturated, pair A's normalize/LN/FFN chain and next-block transposes
run on DVE/PE in the shadow.  ACT (the exp stream) is the roofline engine.
"""

import sys

import numpy as np
import ml_dtypes

if "/opt/trn_rl_repo" not in sys.path:
    sys.path.insert(0, "/opt/trn_rl_repo")

import concourse.bass as bass
import concourse.bacc as bacc
import concourse.tile as tile
from concourse import mybir
from concourse.bass_utils import run_bass_kernel_spmd

F32 = mybir.dt.float32
BF16 = mybir.dt.bfloat16
AF = mybir.ActivationFunctionType
ALU = mybir.AluOpType

# Force Exp and Ln to resolve to the one table set containing both, so the
# ACT engine never thrashes ACT_TABLE_LOADs between them.
_orig_gat = bacc.get_activation_tables
def _gat_patched(arch):
    out = {}
    for name, fns in _orig_gat(arch).items():
        fns = set(fns)
        if name != "natural_log_exp_and_others":
            fns.discard(mybir.ActivationFunctionType.Exp)
            fns.discard(mybir.ActivationFunctionType.Ln)
        out[name] = fns
    return out
bacc.get_activation_tables = _gat_patched

B, N, F_IN, H = 32, 1024, 256, 64
NCORES = 8
S = B // NCORES            # samples per core
NPAIR = S // 2             # sample pairs per core
EPS = 1e-5
SCALE = float(1.0 / np.sqrt(np.float32(N)))
NH = 512                   # free-dim half (PSUM bank)
UDELAY = 5                 # U-matmul issue delay, in (chunk, sample) units


def build_nc(use_mask: bool, use_gb: bool) -> bass.Bass:
    nc = bacc.Bacc("TRN2", target_bir_lowering=False, debug=False, num_devices=NCORES)

    xT = nc.declare_dram_parameter("xT", [S, 2, 128, N], BF16, isOutput=False)
    we = nc.declare_dram_parameter("We", [2, 128, H], BF16, isOutput=False)
    w0 = nc.declare_dram_parameter("W0s", [128, H], BF16, isOutput=False)
    w1 = nc.declare_dram_parameter("W1s", [128, H], BF16, isOutput=False)
    wout = nc.declare_dram_parameter("Wouts", [128, 1], F32, isOutput=False)
    be2 = nc.declare_dram_parameter("be2", [128, 1], F32, isOutput=False)
    b02 = nc.declare_dram_parameter("b02", [128, 1], F32, isOutput=False)
    b12 = nc.declare_dram_parameter("b12", [128, 1], F32, isOutput=False)
    boutp = nc.declare_dram_parameter("bout", [1, 1], F32, isOutput=False)
    ident = nc.declare_dram_parameter("ident", [128, 128], BF16, isOutput=False)
    identf = nc.declare_dram_parameter("identf", [128, 128], F32, isOutput=False)
    selsum_d = nc.declare_dram_parameter("selsum", [128, 2], F32, isOutput=False)
    selbc_d = nc.declare_dram_parameter("selbc", [2, 128], F32, isOutput=False)
    if use_gb:
        gT2_d = nc.declare_dram_parameter("gT2", [128, N], F32, isOutput=False)
        bT2_d = nc.declare_dram_parameter("bT2", [128, N], F32, isOutput=False)
    if use_mask:
        maskN_d = nc.declare_dram_parameter("maskN", [S, N, N], F32, isOutput=False)
    out_d = nc.declare_dram_parameter("out", [S, 1], F32, isOutput=True)

    with tile.TileContext(nc) as tc:
        with (
            tc.tile_pool(name="consts", bufs=1) as cp,
            tc.tile_pool(name="xt", bufs=8) as xp,
            tc.tile_pool(name="big", bufs=2) as bigp,
            tc.tile_pool(name="hn", bufs=2) as hnp,
            tc.tile_pool(name="e", bufs=8) as ep,
            tc.tile_pool(name="small", bufs=2) as smp,
            tc.tile_pool(name="ring", bufs=2, space="PSUM") as pg,
            tc.tile_pool(name="pu", bufs=2, space="PSUM") as pu,
            tc.tile_pool(name="pm", bufs=2, space="PSUM") as pm,
        ):
            # ---- constants ----
            we_sb = cp.tile([128, 2, H], BF16, tag="we", name="we_sb")
            nc.sync.dma_start(we_sb[:, :, :], we.rearrange("k p m -> p k m"))
            w0_sb = cp.tile([128, H], BF16, tag="w0", name="w0_sb")
            nc.sync.dma_start(w0_sb[:, :], w0[:, :])
            w1_sb = cp.tile([128, H], BF16, tag="w1", name="w1_sb")
            nc.sync.dma_start(w1_sb[:, :], w1[:, :])
            wo_sb = cp.tile([128, 1], F32, tag="wo", name="wo_sb")
            nc.sync.dma_start(wo_sb[:, :], wout[:, :])
            be_sb = cp.tile([128, 1], F32, tag="be", name="be_sb")
            nc.sync.dma_start(be_sb[:, :], be2[:, :])
            b0_sb = cp.tile([128, 1], F32, tag="b0", name="b0_sb")
            nc.sync.dma_start(b0_sb[:, :], b02[:, :])
            b1_sb = cp.tile([128, 1], F32, tag="b1", name="b1_sb")
            nc.sync.dma_start(b1_sb[:, :], b12[:, :])
            bo_sb = cp.tile([1, 1], F32, tag="bo", name="bo_sb")
            nc.sync.dma_start(bo_sb[:, :], boutp[:, :])
            id_sb = cp.tile([128, 128], BF16, tag="id", name="id_sb")
            nc.sync.dma_start(id_sb[:, :], ident[:, :])
            idf_sb = cp.tile([128, 128], F32, tag="idf", name="idf_sb")
            nc.sync.dma_start(idf_sb[:, :], identf[:, :])
            selsum = cp.tile([128, 2], F32, tag="ss", name="selsum_sb")
            nc.sync.dma_start(selsum[:, :], selsum_d[:, :])
            selbc = cp.tile([2, 128], F32, tag="sb", name="selbc_sb")
            nc.sync.dma_start(selbc[:, :], selbc_d[:, :])
            eps2 = cp.tile([2, 1], F32, tag="eps", name="eps2")
            nc.vector.memset(eps2[:, :], EPS)
            gb = None
            if use_gb:
                gam = cp.tile([128, N], F32, tag="gam", name="gam")
                nc.sync.dma_start(gam[:, :], gT2_d[:, :])
                bet = cp.tile([128, N], F32, tag="bet", name="bet")
                nc.sync.dma_start(bet[:, :], bT2_d[:, :])
                gb = (gam, bet)

            pair_state = [dict() for _ in range(NPAIR)]

            # ---- LayerNorm pieces ----
            def ln_stats(v, nm):
                """DVE stats chain -> (sv [2,2] = per-sample (mean, ex2),
                var [2,1]).  No ACT ops."""
                st6 = smp.tile([128, 12], F32, tag="st6", name=f"st6_{nm}")
                nc.vector.bn_stats(st6[:, 0:6], v[:, 0:NH])
                nc.vector.bn_stats(st6[:, 6:12], v[:, NH:N])
                ag = smp.tile([128, 2], F32, tag="ag", name=f"ag_{nm}")
                nc.vector.bn_aggr(ag[:, :], st6[:, :])
                m2 = smp.tile([128, 2], F32, tag="m2", name=f"m2_{nm}")
                nc.vector.tensor_copy(m2[:, 0:1], ag[:, 0:1])
                msqp = smp.tile([128, 1], F32, tag="msqp", name=f"msqp_{nm}")
                nc.vector.tensor_tensor(msqp[:, :], ag[:, 0:1], ag[:, 0:1], op=ALU.mult)
                nc.vector.tensor_tensor(m2[:, 1:2], ag[:, 1:2], msqp[:, :], op=ALU.add)
                stps = pm.tile([2, 2], F32, tag="m", name=f"stps_{nm}")
                nc.tensor.matmul(stps[:, :], selsum[:, :], m2[:, :], start=True, stop=True)
                sv = smp.tile([2, 2], F32, tag="sv", name=f"sv_{nm}")
                nc.vector.tensor_copy(sv[:, :], stps[:, :])
                msq = smp.tile([2, 1], F32, tag="msq", name=f"msq_{nm}")
                nc.vector.tensor_tensor(msq[:, :], sv[:, 0:1], sv[:, 0:1], op=ALU.mult)
                var = smp.tile([2, 1], F32, tag="var", name=f"var_{nm}")
                nc.vector.tensor_tensor(var[:, :], sv[:, 1:2], msq[:, :], op=ALU.subtract)
                return sv, var

            def ln_finish(v, sv, var, out, nm):
                """ACT Ln/Exp -> rho, broadcast via PE, apply."""
                lnv = smp.tile([2, 1], F32, tag="lnv", name=f"lnv_{nm}")
                nc.scalar.activation(lnv[:, :], var[:, :], AF.Ln, bias=eps2[:, :])
                pb = smp.tile([2, 2], F32, tag="pb", name=f"pb_{nm}")
                nc.scalar.activation(pb[:, 1:2], lnv[:, :], AF.Exp, scale=-0.5)
                nc.vector.tensor_scalar(pb[:, 0:1], sv[:, 0:1], -1.0, None, op0=ALU.mult)
                bcps = pm.tile([128, 2], F32, tag="m", name=f"bcps_{nm}")
                nc.tensor.matmul(bcps[:, :], selbc[:, :], pb[:, :], start=True, stop=True)
                bc = smp.tile([128, 2], F32, tag="bc", name=f"bc_{nm}")
                nc.vector.tensor_copy(bc[:, :], bcps[:, :])
                nc.vector.tensor_scalar(
                    out[:, :], v[:, :], bc[:, 0:1], bc[:, 1:2],
                    op0=ALU.add, op1=ALU.mult,
                )
                if gb is not None:
                    nc.vector.tensor_tensor(out[:, :], out[:, :], gb[0][:, :], op=ALU.mult)
                    nc.vector.tensor_tensor(out[:, :], out[:, :], gb[1][:, :], op=ALU.add)

            # ---- transposes: hT [128, 1024] -> hnc [128, 8, 128] ----
            def emit_transposes(hT_src, nm):
                tp = pm.tile([128, 8, 128], BF16, tag="m", name=f"tp_{nm}")
                for c in range(8):
                    nc.tensor.transpose(
                        tp[:, c, :], hT_src[:, 128 * c:128 * c + 128], id_sb[:, :],
                    )
                hnc = hnp.tile([128, 8, 128], BF16, tag="hn", name=f"hnc_{nm}")
                nc.vector.tensor_copy(hnc[:, :, :], tp[:, :, :])
                return hnc

            # ---- embed: x_embT = relu(We.T @ xT + be) ----
            def emit_embed(p):
                xts = []
                for si in range(2):
                    for k in range(2):
                        t = xp.tile([128, N], BF16, tag="xt", name=f"x_{p}_{si}_{k}")
                        nc.sync.dma_start(t[:, :], xT[2 * p + si, k, :, :])
                        xts.append((si, k, t))
                xe = bigp.tile([128, N], BF16, tag="xemb", name=f"xe_{p}")
                for half in range(2):
                    cols = slice(NH * half, NH * half + NH)
                    emb = pm.tile([128, NH], F32, tag="m", name=f"emb_{p}_{half}")
                    for (si, k, t) in xts:
                        nc.tensor.matmul(
                            emb[64 * si:64 * si + 64, :], we_sb[:, k, :], t[:, cols],
                            start=(k == 0), stop=(k == 1),
                            tile_position=(0, 64 * si), skip_group_check=True,
                        )
                    nc.vector.tensor_scalar(
                        xe[:, cols], emb[:, :], be_sb[:, :], 0.0,
                        op0=ALU.add, op1=ALU.max,
                    )
                pair_state[p]["xemb"] = xe
                pair_state[p]["hT"] = xe

            # ---- attention phase for (pair, block) ----
            def emit_attention(p, b, hooks):
                st = pair_state[p]
                hT = st["hT"]
                hnc = st["hnc"]
                zacc = smp.tile([128, 16], F32, tag="zacc", name=f"zacc_{p}_{b}")
                u_lo = pu.tile([128, NH], F32, tag="u", name=f"ulo_{p}_{b}")
                u_hi = pu.tile([128, NH], F32, tag="u", name=f"uhi_{p}_{b}")
                pend = []

                def emit_u(item):
                    c, s, e = item
                    r0 = 64 * s
                    nc.tensor.matmul(
                        u_lo[r0:r0 + 64, :], hnc[:, c, r0:r0 + 64], e[:, 0:NH],
                        start=(c == 0), stop=(c == 7), skip_group_check=True,
                    )
                    nc.tensor.matmul(
                        u_hi[r0:r0 + 64, :], hnc[:, c, r0:r0 + 64], e[:, NH:N],
                        start=(c == 0), stop=(c == 7), skip_group_check=True,
                    )

                for c in range(8):
                    for s in range(2):
                        g = pg.tile([128, N], F32, tag="g", name=f"g_{p}_{b}_{c}_{s}")
                        r0 = 64 * s
                        lhs = hT[r0:r0 + 64, 128 * c:128 * c + 128]
                        nc.tensor.matmul(g[:, 0:NH], lhs, hT[r0:r0 + 64, 0:NH],
                                         start=True, stop=True)
                        nc.tensor.matmul(g[:, NH:N], lhs, hT[r0:r0 + 64, NH:N],
                                         start=True, stop=True)
                        if use_mask:
                            mt = ep.tile([128, N], F32, tag="mt", bufs=2,
                                         name=f"mt_{p}_{b}_{c}_{s}")
                            nc.sync.dma_start(
                                mt[:, :],
                                maskN_d[2 * p + s, 128 * c:128 * c + 128, :],
                            )
                            nc.vector.tensor_tensor(g[:, :], g[:, :], mt[:, :],
                                                    op=ALU.add)
                        e = ep.tile([128, N], BF16, tag="e", bufs=8,
                                    name=f"e_{p}_{b}_{c}_{s}")
                        col = 8 * s + c
                        nc.scalar.activation(
                            e[:, :], g[:, :], AF.Exp, scale=SCALE,
                            accum_out=zacc[:, col:col + 1],
                        )
                        pend.append((c, s, e))
                        while len(pend) > UDELAY:
                            emit_u(pend.pop(0))
                    if c in hooks:
                        hooks[c]()
                while pend:
                    emit_u(pend.pop(0))
                return zacc, u_lo, u_hi

            # ---- post-attention chain, split for interleaved emission ----
            def make_post(p, b, zacc, u_lo, u_hi):
                st = {}
                wf = w0_sb if b == 0 else w1_sb
                bf = b0_sb if b == 0 else b1_sb

                def a1():
                    # 1/Z and its journey to row form (no ACT ops)
                    rzc = smp.tile([128, 16], F32, tag="rzc", name=f"rzc_{p}_{b}")
                    nc.vector.reciprocal(rzc[:, :], zacc[:, :])
                    zrt = pm.tile([16, 128], F32, tag="m", name=f"zrt_{p}_{b}")
                    nc.tensor.transpose(zrt[:, :], rzc[:, :], idf_sb[:, :])
                    rzt = smp.tile([16, 128], F32, tag="rzt", name=f"rzt_{p}_{b}")
                    nc.vector.tensor_copy(rzt[:, :], zrt[:, :])
                    rzrow = smp.tile([2, 8, 128], F32, tag="rzrow",
                                     name=f"rzrow_{p}_{b}")
                    nc.sync.dma_start(rzrow[0:1, :, :], rzt[0:8, :])
                    nc.sync.dma_start(rzrow[1:2, :, :], rzt[8:16, :])
                    st["rzrow"] = rzrow

                def a2():
                    # broadcast 1/Z across partitions, normalize U, residual,
                    # then qkv LN stats (no ACT ops)
                    rzrow2 = st["rzrow"].rearrange("s c j -> s (c j)")
                    rzb_lo = pm.tile([128, NH], F32, tag="m", name=f"rzblo_{p}_{b}")
                    rzb_hi = pm.tile([128, NH], F32, tag="m", name=f"rzbhi_{p}_{b}")
                    nc.tensor.matmul(rzb_lo[:, :], selbc[:, :], rzrow2[:, 0:NH],
                                     start=True, stop=True)
                    nc.tensor.matmul(rzb_hi[:, :], selbc[:, :], rzrow2[:, NH:N],
                                     start=True, stop=True)
                    # DVE reads at most one PSUM operand per instruction, so
                    # stage 1/Z in SBUF before the U multiply.
                    rzb = smp.tile([128, N], F32, tag="rzb", name=f"rzb_{p}_{b}")
                    nc.vector.tensor_copy(rzb[:, 0:NH], rzb_lo[:, :])
                    nc.vector.tensor_copy(rzb[:, NH:N], rzb_hi[:, :])
                    v = bigp.tile([128, N], BF16, tag="v", name=f"v_{p}_{b}")
                    nc.vector.tensor_tensor(v[:, 0:NH], u_lo[:, :], rzb[:, 0:NH],
                                            op=ALU.mult)
                    nc.vector.tensor_tensor(v[:, NH:N], u_hi[:, :], rzb[:, NH:N],
                                            op=ALU.mult)
                    nc.vector.tensor_tensor(v[:, :], v[:, :],
                                            pair_state[p]["xemb"][:, :], op=ALU.add)
                    st["v"] = v
                    st["sv1"], st["var1"] = ln_stats(v, f"q{p}{b}")

                def bfn():
                    # qkv LN finish (2 tiny ACT ops), FFN, fc LN stats
                    qkv = bigp.tile([128, N], BF16, tag="qkv", name=f"qkv_{p}_{b}")
                    ln_finish(st["v"], st["sv1"], st["var1"], qkv, f"q{p}{b}")
                    fps_lo = pm.tile([128, NH], F32, tag="m", name=f"fpslo_{p}_{b}")
                    fps_hi = pm.tile([128, NH], F32, tag="m", name=f"fpshi_{p}_{b}")
                    for fps, cols in ((fps_lo, slice(0, NH)), (fps_hi, slice(NH, N))):
                        nc.tensor.matmul(fps[0:64, :], wf[0:64, :], qkv[0:64, cols],
                                         start=True, stop=True)
                        nc.tensor.matmul(fps[64:128, :], wf[64:128, :],
                                         qkv[64:128, cols], start=True, stop=True)
                    f = bigp.tile([128, N], BF16, tag="f", name=f"f_{p}_{b}")
                    nc.vector.tensor_scalar(f[:, 0:NH], fps_lo[:, :], bf[:, :], 0.0,
                                            op0=ALU.add, op1=ALU.max)
                    nc.vector.tensor_scalar(f[:, NH:N], fps_hi[:, :], bf[:, :], 0.0,
                                            op0=ALU.add, op1=ALU.max)
                    nc.vector.tensor_tensor(f[:, :], f[:, :], qkv[:, :], op=ALU.add)
                    st["f"] = f
                    st["sv2"], st["var2"] = ln_stats(f, f"f{p}{b}")

                def cfn():
                    # fc LN finish (2 tiny ACT ops), then either next-block
                    # transposes or the output head
                    fc = bigp.tile([128, N], BF16, tag="fc", name=f"fc_{p}_{b}")
                    ln_finish(st["f"], st["sv2"], st["var2"], fc, f"f{p}{b}")
                    pair_state[p]["hT"] = fc
                    if b == 0:
                        pair_state[p]["hnc"] = emit_transposes(fc, f"{p}_{b + 1}")
                    else:
                        emit_head(p, fc)

                return a1, a2, bfn, cfn

            # ---- pool + head ----
            def emit_head(p, fc):
                pooled = smp.tile([128, 1], F32, tag="pool", name=f"pool_{p}")
                nc.vector.reduce_sum(pooled[:, :], fc[:, :],
                                     axis=mybir.AxisListType.X)
                sc = pm.tile([1, 2], F32, tag="m", name=f"sc_{p}")
                nc.tensor.matmul(sc[0:1, 0:1], wo_sb[0:64, :], pooled[0:64, :],
                                 start=True, stop=True, skip_group_check=True)
                nc.tensor.matmul(sc[0:1, 1:2], wo_sb[64:128, :], pooled[64:128, :],
                                 start=True, stop=True, skip_group_check=True)
                zb = smp.tile([1, 2], F32, tag="zb", name=f"zb_{p}")
                nc.vector.tensor_scalar(zb[:, :], sc[:, :], bo_sb[0:1, :], None,
                                        op0=ALU.add)
                res = smp.tile([1, 2], F32, tag="res", name=f"res_{p}")
                nc.vector.tensor_scalar(res[:, :], zb[:, :], 0.01, None,
                                        op0=ALU.mult)
                nc.vector.tensor_tensor(res[:, :], res[:, :], zb[:, :], op=ALU.max)
                for si in range(2):
                    nc.sync.dma_start(
                        out_d[2 * p + si:2 * p + si + 1, :], res[0:1, si:si + 1],
                    )

            # ---- main schedule ----
            emit_embed(0)
            pair_state[0]["hnc"] = emit_transposes(pair_state[0]["hT"], "0_0")
            emit_embed(1)

            phases = [(0, 0), (1, 0), (0, 1), (1, 1)]
            pending = None
            for i, (p, b) in enumerate(phases):
                hooks = {}
                if pending is not None:
                    a1, a2, bfn, cfn = pending
                    hooks[0] = a1
                    hooks[1] = a2
                    hooks[3] = bfn
                    hooks[5] = cfn
                if i == 0:
                    hooks[2] = lambda: pair_state[1].__setitem__(
                        "hnc", emit_transposes(pair_state[1]["hT"], "1_0"))
                zacc, u_lo, u_hi = emit_attention(p, b, hooks)
                pending = make_post(p, b, zacc, u_lo, u_hi)

            for fn in pending:
                fn()

    nc.compile()
    return nc


_NC_CACHE: dict = {}


def prepare_common(We, be, gamma, beta, W0, b0, W1, b1, Wout, bout, use_gb):
    ident = np.eye(128, dtype=np.float32)
    selsum = np.zeros((128, 2), dtype=np.float32)
    selsum[0:64, 0] = 1.0 / 64.0
    selsum[64:128, 1] = 1.0 / 64.0
    selbc = np.zeros((2, 128), dtype=np.float32)
    selbc[0, 0:64] = 1.0
    selbc[1, 64:128] = 1.0

    def stack2(v):
        v = np.asarray(v, dtype=np.float32).reshape(-1)
        return np.concatenate([v, v]).reshape(128, 1)

    common = {
        "We": np.ascontiguousarray(np.asarray(We, dtype=np.float32)).reshape(
            2, 128, H).astype(ml_dtypes.bfloat16),
        "W0s": np.concatenate([W0, W0]).astype(ml_dtypes.bfloat16),
        "W1s": np.concatenate([W1, W1]).astype(ml_dtypes.bfloat16),
        "Wouts": np.concatenate([Wout, Wout]).astype(np.float32),
        "be2": stack2(be), "b02": stack2(b0), "b12": stack2(b1),
        "bout": np.asarray(bout, dtype=np.float32).reshape(1, 1),
        "ident": ident.astype(ml_dtypes.bfloat16),
        "identf": ident.astype(np.float32),
        "selsum": selsum, "selbc": selbc,
    }
    if use_gb:
        gT = np.ascontiguousarray(np.asarray(gamma, dtype=np.float32).T)
        bT = np.ascontiguousarray(np.asarray(beta, dtype=np.float32).T)
        common["gT2"] = np.concatenate([gT, gT]).astype(np.float32)
        common["bT2"] = np.concatenate([bT, bT]).astype(np.float32)
    return common


def prepare_core_map(common, x, mask, k, use_mask):
    xs = x[S * k:S * k + S]                       # [S, N, F_IN]
    xTs = np.ascontiguousarray(xs.transpose(0, 2, 1)).reshape(S, 2, 128, N)
    m = dict(common)
    m["xT"] = xTs.astype(ml_dtypes.bfloat16)
    if use_mask:
        m["maskN"] = np.ascontiguousarray(mask[S * k:S * k + S])
    return m


def kernel(x, mask, We, be, gamma, beta, W0, b0, W1, b1, Wout, bout):
    x = np.ascontiguousarray(np.asarray(x, dtype=np.float32))
    mask = np.asarray(mask, dtype=np.float32)
    use_mask = bool(np.any(mask))
    use_gb = bool(np.any(np.asarray(gamma) != 1.0) or np.any(np.asarray(beta)))

    key = (use_mask, use_gb)
    if key not in _NC_CACHE:
        _NC_CACHE[key] = build_nc(use_mask, use_gb)
    nc = _NC_CACHE[key]

    common = prepare_common(We, be, gamma, beta, W0, b0, W1, b1, Wout, bout,
                            use_gb)
    in_maps = [prepare_core_map(common, x, mask, k, use_mask)
               for k in range(NCORES)]

    res = run_bass_kernel_spmd(nc, in_maps, list(range(NCORES)))
    global LAST_RESULT
    LAST_RESULT = res
    out = np.concatenate([res.results[k]["out"] for k in range(NCORES)], axis=0)
    return out.astype(np.float32)


LAST_RESULT = None


# revision 21
# speedup vs baseline: 1.5100x; 1.0447x over previous
"""Trainium2 Bass kernel for nn_HLSTransformer (2-block self-attention encoder).

Contract: kernel(**inputs) takes FULL inputs, returns FULL output [32, 1] f32.
Data-parallel over batch: 4 samples per core x 8 cores.

On-device layout: activations in "T layout" [H=64 partitions, N=1024 free],
two samples packed per 128-partition tile (rows 0-63 = even sample, 64-127 =
odd sample of the pair).

Attention design (per pair of samples, per block):
  - For each n-chunk c (8 chunks of 128 tokens) and sample s, one PSUM gram
    tile g = S_s[n in chunk, m in 0:1024] via two row-tiled matmuls (the two
    samples' matmuls run concurrently on disjoint PE row strips).
  - One big ACT exp per tile writes e (bf16 SBUF) and, via accum_out, the
    per-row partial sums = softmax denominators Z (scores are symmetric, so
    row sums equal the column sums needed later).  No max-subtract needed:
    softmax((s-max+mask)*SCALE) == softmax((s+mask)*SCALE) and |s*SCALE| is
    small enough that fp32 exp cannot overflow.
  - U = E @ h accumulated over chunks into two PSUM banks (m-halves); the two
    samples' U matmuls are column-tiled onto disjoint PE col strips and run
    concurrently.  U issue is delayed a few chunks so the previous pair's
    normalize can release the U banks without stalling the PE queue.
  - 1/Z: DVE reciprocal -> PE transpose -> SBUF-to-SBUF DMA gather into row
    form [2, 1024] -> one matmul pair against a {0,1} selector broadcasts it
    across partitions -> DVE multiply normalizes U.
  - LayerNorm over the whole [N, H] slab per sample -> scalar mean/var;
    rsqrt(var+eps) = exp(-0.5*ln(var+eps)) keeps ACT on one table set.

Emission interleaves the two pairs: while pair B's exp stream keeps the ACT
engine saturated, pair A's normalize/LN/FFN chain and next-block transposes
run on DVE/PE in the shadow.  ACT (the exp stream) is the roofline engine.
"""

import sys

import numpy as np
import ml_dtypes

if "/opt/trn_rl_repo" not in sys.path:
    sys.path.insert(0, "/opt/trn_rl_repo")

import concourse.bass as bass
import concourse.bacc as bacc
import concourse.tile as tile
from concourse import mybir
from concourse.bass_utils import run_bass_kernel_spmd

F32 = mybir.dt.float32
DT16 = mybir.dt.float16
FP8 = mybir.dt.float8e4
AF = mybir.ActivationFunctionType
ALU = mybir.AluOpType

USE_FP8_U = False          # fp8 DoubleRow for the U (= E @ h) matmuls

# Force Exp and Ln to resolve to the one table set containing both, so the
# ACT engine never thrashes ACT_TABLE_LOADs between them.
_orig_gat = bacc.get_activation_tables
def _gat_patched(arch):
    out = {}
    for name, fns in _orig_gat(arch).items():
        fns = set(fns)
        if name != "natural_log_exp_and_others":
            fns.discard(mybir.ActivationFunctionType.Exp)
            fns.discard(mybir.ActivationFunctionType.Ln)
        out[name] = fns
    return out
bacc.get_activation_tables = _gat_patched

B, N, F_IN, H = 32, 1024, 256, 64
NCORES = 8
S = B // NCORES            # samples per core
NPAIR = S // 2             # sample pairs per core
EPS = 1e-5
SCALE = float(1.0 / np.sqrt(np.float32(N)))
NH = 512                   # free-dim half (PSUM bank)
UDELAY = 3                 # U-matmul issue delay, in (chunk-pair, sample) units


def build_nc(use_mask: bool, use_gb: bool) -> bass.Bass:
    nc = bacc.Bacc("TRN2", target_bir_lowering=False, debug=False, num_devices=NCORES)

    EDT = FP8 if USE_FP8_U else DT16
    xT = nc.declare_dram_parameter("xT", [S, 2, 128, N], DT16, isOutput=False)
    we = nc.declare_dram_parameter("We", [2, 128, H], DT16, isOutput=False)
    w0 = nc.declare_dram_parameter("W0s", [128, H], DT16, isOutput=False)
    w1 = nc.declare_dram_parameter("W1s", [128, H], DT16, isOutput=False)
    wout = nc.declare_dram_parameter("Wouts", [128, 1], F32, isOutput=False)
    be2 = nc.declare_dram_parameter("be2", [128, 1], F32, isOutput=False)
    b02 = nc.declare_dram_parameter("b02", [128, 1], F32, isOutput=False)
    b12 = nc.declare_dram_parameter("b12", [128, 1], F32, isOutput=False)
    boutp = nc.declare_dram_parameter("bout", [1, 1], F32, isOutput=False)
    ident = nc.declare_dram_parameter("ident", [128, 128], DT16, isOutput=False)
    selbcb_d = nc.declare_dram_parameter("selbcb", [2, 128], DT16, isOutput=False)
    allsel_d = nc.declare_dram_parameter("allsel", [128, 128], F32, isOutput=False)
    if use_gb:
        gT2_d = nc.declare_dram_parameter("gT2", [128, N], F32, isOutput=False)
        bT2_d = nc.declare_dram_parameter("bT2", [128, N], F32, isOutput=False)
    if use_mask:
        maskN_d = nc.declare_dram_parameter("maskN", [S, N, N], F32, isOutput=False)
    out_d = nc.declare_dram_parameter("out", [S, 1], F32, isOutput=True)

    with tile.TileContext(nc) as tc:
        with (
            tc.tile_pool(name="consts", bufs=1) as cp,
            tc.tile_pool(name="xt", bufs=8) as xp,
            tc.tile_pool(name="big", bufs=2) as bigp,
            tc.tile_pool(name="hn", bufs=2) as hnp,
            tc.tile_pool(name="e", bufs=8) as ep,
            tc.tile_pool(name="small", bufs=2) as smp,
            tc.tile_pool(name="ring", bufs=2, space="PSUM") as pg,
            tc.tile_pool(name="pu", bufs=2, space="PSUM") as pu,
            tc.tile_pool(name="pm", bufs=2, space="PSUM") as pm,
        ):
            # ---- constants (embed-critical ones first for startup) ----
            we_sb = cp.tile([128, 2, H], DT16, tag="we", name="we_sb")
            nc.sync.dma_start(we_sb[:, :, :], we.rearrange("k p m -> p k m"))
            be_sb = cp.tile([128, 1], F32, tag="be", name="be_sb")
            nc.sync.dma_start(be_sb[:, :], be2[:, :])
            id_sb = cp.tile([128, 128], DT16, tag="id", name="id_sb")
            nc.sync.dma_start(id_sb[:, :], ident[:, :])
            w0_sb = cp.tile([128, H], DT16, tag="w0", name="w0_sb")
            nc.sync.dma_start(w0_sb[:, :], w0[:, :])
            w1_sb = cp.tile([128, H], DT16, tag="w1", name="w1_sb")
            nc.sync.dma_start(w1_sb[:, :], w1[:, :])
            wo_sb = cp.tile([128, 1], F32, tag="wo", name="wo_sb")
            nc.sync.dma_start(wo_sb[:, :], wout[:, :])
            b0_sb = cp.tile([128, 1], F32, tag="b0", name="b0_sb")
            nc.sync.dma_start(b0_sb[:, :], b02[:, :])
            b1_sb = cp.tile([128, 1], F32, tag="b1", name="b1_sb")
            nc.sync.dma_start(b1_sb[:, :], b12[:, :])
            bo_sb = cp.tile([1, 1], F32, tag="bo", name="bo_sb")
            nc.sync.dma_start(bo_sb[:, :], boutp[:, :])
            selbcb = cp.tile([2, 128], DT16, tag="sbb", name="selbcb_sb")
            nc.sync.dma_start(selbcb[:, :], selbcb_d[:, :])
            allsel = cp.tile([128, 128], F32, tag="asel", name="allsel_sb")
            nc.sync.dma_start(allsel[:, :], allsel_d[:, :])
            eps128 = cp.tile([128, 1], F32, tag="eps", name="eps128")
            nc.vector.memset(eps128[:, :], EPS)
            gb = None
            if use_gb:
                gam = cp.tile([128, N], F32, tag="gam", name="gam")
                nc.sync.dma_start(gam[:, :], gT2_d[:, :])
                bet = cp.tile([128, N], F32, tag="bet", name="bet")
                nc.sync.dma_start(bet[:, :], bT2_d[:, :])
                gb = (gam, bet)

            pair_state = [dict() for _ in range(NPAIR)]

            # ---- LayerNorm pieces ----
            # Per-sample scalar mean/var over the [N, H] slab.  Per-partition
            # (sum, sqsum) on DVE, then ONE matmul against a constant
            # block-diagonal (1/65536)-matrix performs the per-sample-group
            # partition reduce, the broadcast, and the averaging at once.
            def ln_stats(v, nm):
                zs = smp.tile([128, 2], F32, tag="zs", name=f"zs_{nm}")
                nc.vector.reduce_sum(zs[:, 0:1], v[:, :], axis=mybir.AxisListType.X)
                scr = bigp.tile([128, N], DT16, tag="scr", name=f"scr_{nm}")
                nc.vector.scalar_tensor_tensor(
                    scr[:, :], v[:, :], 1.0, v[:, :],
                    op0=ALU.mult, op1=ALU.mult, accum_out=zs[:, 1:2],
                )
                mrp = pm.tile([128, 2], F32, tag="m", name=f"mrp_{nm}")
                nc.tensor.matmul(mrp[:, :], allsel[:, :], zs[:, :],
                                 start=True, stop=True)
                mrs = smp.tile([128, 2], F32, tag="mrs", name=f"mrs_{nm}")
                nc.vector.tensor_copy(mrs[:, :], mrp[:, :])
                return mrs

            def ln_finish(v, mrs, out, nm):
                """nvar = mean^2 - ex2 (= -var); rho = exp(-0.5*ln(var+eps));
                out = (v - mean) * rho.  Two tiny ACT ops."""
                nvar = smp.tile([128, 1], F32, tag="nvar", name=f"nvar_{nm}")
                nc.vector.scalar_tensor_tensor(
                    nvar[:, :], mrs[:, 0:1], mrs[:, 0:1], mrs[:, 1:2],
                    op0=ALU.mult, op1=ALU.subtract,
                )
                lnv = smp.tile([128, 1], F32, tag="lnv", name=f"lnv_{nm}")
                nc.scalar.activation(lnv[:, :], nvar[:, :], AF.Ln, scale=-1.0,
                                     bias=eps128[:, :])
                rho = smp.tile([128, 1], F32, tag="rho", name=f"rho_{nm}")
                nc.scalar.activation(rho[:, :], lnv[:, :], AF.Exp, scale=-0.5)
                nc.vector.tensor_scalar(
                    out[:, :], v[:, :], mrs[:, 0:1], rho[:, 0:1],
                    op0=ALU.subtract, op1=ALU.mult,
                )
                if gb is not None:
                    nc.vector.tensor_tensor(out[:, :], out[:, :], gb[0][:, :], op=ALU.mult)
                    nc.vector.tensor_tensor(out[:, :], out[:, :], gb[1][:, :], op=ALU.add)

            # ---- transposes: hT [128, 1024] -> hnc [128, 8, 128] ----
            def emit_transposes(hT_src, nm):
                tp = pm.tile([128, 8, 128], DT16, tag="m", name=f"tp_{nm}")
                for c in range(8):
                    nc.tensor.transpose(
                        tp[:, c, :], hT_src[:, 128 * c:128 * c + 128], id_sb[:, :],
                    )
                hnc = hnp.tile([128, 8, 128], EDT, tag="hn", name=f"hnc_{nm}")
                nc.vector.tensor_copy(hnc[:, :, :], tp[:, :, :])
                return hnc

            # ---- embed: x_embT = relu(We.T @ xT + be) ----
            def emit_embed(p):
                xts = []
                for si in range(2):
                    for k in range(2):
                        t = xp.tile([128, N], DT16, tag="xt", name=f"x_{p}_{si}_{k}")
                        nc.sync.dma_start(t[:, :], xT[2 * p + si, k, :, :])
                        xts.append((si, k, t))
                xe = bigp.tile([128, N], DT16, tag="xemb", name=f"xe_{p}")
                for half in range(2):
                    cols = slice(NH * half, NH * half + NH)
                    emb = pm.tile([128, NH], F32, tag="m", name=f"emb_{p}_{half}")
                    for (si, k, t) in xts:
                        nc.tensor.matmul(
                            emb[64 * si:64 * si + 64, :], we_sb[:, k, :], t[:, cols],
                            start=(k == 0), stop=(k == 1),
                            tile_position=(0, 64 * si), skip_group_check=True,
                        )
                    nc.vector.tensor_scalar(
                        xe[:, cols], emb[:, :], be_sb[:, :], 0.0,
                        op0=ALU.add, op1=ALU.max,
                    )
                pair_state[p]["xemb"] = xe
                pair_state[p]["hT"] = xe

            # ---- attention phase for (pair, block) ----
            def emit_attention(p, b, hooks):
                st = pair_state[p]
                hT = st["hT"]
                hnc = st["hnc"]
                zacc = smp.tile([128, 16], F32, tag="zacc", name=f"zacc_{p}_{b}")
                u_lo = pu.tile([128, NH], F32, tag="u", name=f"ulo_{p}_{b}")
                u_hi = pu.tile([128, NH], F32, tag="u", name=f"uhi_{p}_{b}")
                pend = []

                def emit_u(item):
                    # one DoubleRow matmul contracts a pair of n-chunks
                    cp_, s, e2 = item
                    r0 = 64 * s
                    lhsT = hnc[:, 2 * cp_:2 * cp_ + 2, r0:r0 + 64]
                    for u, half in ((u_lo, slice(0, NH)), (u_hi, slice(NH, N))):
                        if USE_FP8_U:
                            nc.tensor.matmul(
                                u[r0:r0 + 64, :], lhsT, e2[:, :, half],
                                start=(cp_ == 0), stop=(cp_ == 3),
                                perf_mode=mybir.MatmulPerfMode.DoubleRow,
                                skip_group_check=True,
                            )
                        else:
                            for o in range(2):
                                nc.tensor.matmul(
                                    u[r0:r0 + 64, :], hnc[:, 2 * cp_ + o, r0:r0 + 64],
                                    e2[:, o, half],
                                    start=(cp_ == 0 and o == 0),
                                    stop=(cp_ == 3 and o == 1),
                                    skip_group_check=True,
                                )

                e2cur = [None, None]
                for c in range(8):
                    for s in range(2):
                        g = pg.tile([128, N], F32, tag="g", name=f"g_{p}_{b}_{c}_{s}")
                        r0 = 64 * s
                        lhs = hT[r0:r0 + 64, 128 * c:128 * c + 128]
                        nc.tensor.matmul(g[:, 0:NH], lhs, hT[r0:r0 + 64, 0:NH],
                                         start=True, stop=True)
                        nc.tensor.matmul(g[:, NH:N], lhs, hT[r0:r0 + 64, NH:N],
                                         start=True, stop=True)
                        if use_mask:
                            mt = ep.tile([128, N], F32, tag="mt", bufs=2,
                                         name=f"mt_{p}_{b}_{c}_{s}")
                            nc.sync.dma_start(
                                mt[:, :],
                                maskN_d[2 * p + s, 128 * c:128 * c + 128, :],
                            )
                            nc.vector.tensor_tensor(g[:, :], g[:, :], mt[:, :],
                                                    op=ALU.add)
                        if c % 2 == 0:
                            e2cur[s] = ep.tile([128, 2, N], EDT, tag="e", bufs=8,
                                               name=f"e_{p}_{b}_{c}_{s}")
                        e2 = e2cur[s]
                        col = 8 * s + c
                        nc.scalar.activation(
                            e2[:, c % 2, :], g[:, :], AF.Exp, scale=SCALE,
                            accum_out=zacc[:, col:col + 1],
                        )
                        if c % 2 == 1:
                            pend.append((c // 2, s, e2))
                            while len(pend) > UDELAY:
                                emit_u(pend.pop(0))
                    if c in hooks:
                        hooks[c]()
                while pend:
                    emit_u(pend.pop(0))
                return zacc, u_lo, u_hi

            # ---- post-attention chain, split for interleaved emission ----
            def make_post(p, b, zacc, u_lo, u_hi):
                st = {}
                wf = w0_sb if b == 0 else w1_sb
                bf = b0_sb if b == 0 else b1_sb

                def a1():
                    # 1/Z and its journey to row form (no ACT ops)
                    with nc.allow_low_precision("1/Z in bf16 is plenty"):
                        rzc = smp.tile([128, 16], DT16, tag="rzc",
                                       name=f"rzc_{p}_{b}")
                        nc.vector.reciprocal(rzc[:, :], zacc[:, :])
                    zrt = pm.tile([16, 128], DT16, tag="m", name=f"zrt_{p}_{b}")
                    nc.tensor.transpose(zrt[:, :], rzc[:, :], id_sb[:, :])
                    rzt = smp.tile([16, 128], DT16, tag="rzt", name=f"rzt_{p}_{b}")
                    nc.vector.tensor_copy(rzt[:, :], zrt[:, :])
                    rzrow = smp.tile([2, 8, 128], DT16, tag="rzrow",
                                     name=f"rzrow_{p}_{b}")
                    nc.sync.dma_start(rzrow[0:1, :, :], rzt[0:8, :])
                    nc.sync.dma_start(rzrow[1:2, :, :], rzt[8:16, :])
                    st["rzrow"] = rzrow

                def a2():
                    # broadcast 1/Z across partitions, normalize U, residual,
                    # then qkv LN stats (no ACT ops)
                    rzrow2 = st["rzrow"].rearrange("s c j -> s (c j)")
                    rzb_lo = pm.tile([128, NH], F32, tag="m", name=f"rzblo_{p}_{b}")
                    rzb_hi = pm.tile([128, NH], F32, tag="m", name=f"rzbhi_{p}_{b}")
                    nc.tensor.matmul(rzb_lo[:, :], selbcb[:, :], rzrow2[:, 0:NH],
                                     start=True, stop=True)
                    nc.tensor.matmul(rzb_hi[:, :], selbcb[:, :], rzrow2[:, NH:N],
                                     start=True, stop=True)
                    # DVE reads at most one PSUM operand per instruction, so
                    # stage 1/Z in SBUF before the U multiply.
                    rzb = smp.tile([128, N], F32, tag="rzb", name=f"rzb_{p}_{b}")
                    nc.vector.tensor_copy(rzb[:, 0:NH], rzb_lo[:, :])
                    nc.vector.tensor_copy(rzb[:, NH:N], rzb_hi[:, :])
                    v = bigp.tile([128, N], DT16, tag="v", name=f"v_{p}_{b}")
                    nc.vector.tensor_tensor(v[:, 0:NH], u_lo[:, :], rzb[:, 0:NH],
                                            op=ALU.mult)
                    nc.vector.tensor_tensor(v[:, NH:N], u_hi[:, :], rzb[:, NH:N],
                                            op=ALU.mult)
                    nc.vector.tensor_tensor(v[:, :], v[:, :],
                                            pair_state[p]["xemb"][:, :], op=ALU.add)
                    st["v"] = v
                    st["mrs1"] = ln_stats(v, f"q{p}{b}")

                def bfn():
                    # qkv LN finish (2 tiny ACT ops), FFN, fc LN stats
                    qkv = bigp.tile([128, N], DT16, tag="qkv", name=f"qkv_{p}_{b}")
                    ln_finish(st["v"], st["mrs1"], qkv, f"q{p}{b}")
                    fps_lo = pm.tile([128, NH], F32, tag="m", name=f"fpslo_{p}_{b}")
                    fps_hi = pm.tile([128, NH], F32, tag="m", name=f"fpshi_{p}_{b}")
                    for fps, cols in ((fps_lo, slice(0, NH)), (fps_hi, slice(NH, N))):
                        nc.tensor.matmul(fps[0:64, :], wf[0:64, :], qkv[0:64, cols],
                                         start=True, stop=True)
                        nc.tensor.matmul(fps[64:128, :], wf[64:128, :],
                                         qkv[64:128, cols], start=True, stop=True)
                    f = bigp.tile([128, N], DT16, tag="f", name=f"f_{p}_{b}")
                    nc.vector.tensor_scalar(f[:, 0:NH], fps_lo[:, :], bf[:, :], 0.0,
                                            op0=ALU.add, op1=ALU.max)
                    nc.vector.tensor_scalar(f[:, NH:N], fps_hi[:, :], bf[:, :], 0.0,
                                            op0=ALU.add, op1=ALU.max)
                    nc.vector.tensor_tensor(f[:, :], f[:, :], qkv[:, :], op=ALU.add)
                    st["f"] = f
                    st["mrs2"] = ln_stats(f, f"f{p}{b}")

                def cfn():
                    # fc LN finish (2 tiny ACT ops), then either next-block
                    # transposes or the output head
                    fc = bigp.tile([128, N], DT16, tag="fc", name=f"fc_{p}_{b}")
                    ln_finish(st["f"], st["mrs2"], fc, f"f{p}{b}")
                    pair_state[p]["hT"] = fc
                    if b == 0:
                        pair_state[p]["hnc"] = emit_transposes(fc, f"{p}_{b + 1}")
                    else:
                        emit_head(p, fc)

                return a1, a2, bfn, cfn

            # ---- pool + head ----
            def emit_head(p, fc):
                pooled = smp.tile([128, 1], F32, tag="pool", name=f"pool_{p}")
                nc.vector.reduce_sum(pooled[:, :], fc[:, :],
                                     axis=mybir.AxisListType.X)
                sc = pm.tile([1, 2], F32, tag="m", name=f"sc_{p}")
                nc.tensor.matmul(sc[0:1, 0:1], wo_sb[0:64, :], pooled[0:64, :],
                                 start=True, stop=True, skip_group_check=True)
                nc.tensor.matmul(sc[0:1, 1:2], wo_sb[64:128, :], pooled[64:128, :],
                                 start=True, stop=True, skip_group_check=True)
                zb = smp.tile([1, 2], F32, tag="zb", name=f"zb_{p}")
                nc.vector.tensor_scalar(zb[:, :], sc[:, :], bo_sb[0:1, :], None,
                                        op0=ALU.add)
                res = smp.tile([1, 2], F32, tag="res", name=f"res_{p}")
                nc.vector.tensor_scalar(res[:, :], zb[:, :], 0.01, None,
                                        op0=ALU.mult)
                nc.vector.tensor_tensor(res[:, :], res[:, :], zb[:, :], op=ALU.max)
                for si in range(2):
                    nc.sync.dma_start(
                        out_d[2 * p + si:2 * p + si + 1, :], res[0:1, si:si + 1],
                    )

            # ---- main schedule ----
            emit_embed(0)
            pair_state[0]["hnc"] = emit_transposes(pair_state[0]["hT"], "0_0")
            emit_embed(1)

            phases = [(0, 0), (1, 0), (0, 1), (1, 1)]
            pending = None
            for i, (p, b) in enumerate(phases):
                hooks = {}
                if pending is not None:
                    a1, a2, bfn, cfn = pending
                    hooks[0] = a1
                    hooks[1] = a2
                    hooks[3] = bfn
                    hooks[5] = cfn
                if i == 0:
                    hooks[2] = lambda: pair_state[1].__setitem__(
                        "hnc", emit_transposes(pair_state[1]["hT"], "1_0"))
                zacc, u_lo, u_hi = emit_attention(p, b, hooks)
                pending = make_post(p, b, zacc, u_lo, u_hi)

            for fn in pending:
                fn()

    nc.compile()
    return nc


_NC_CACHE: dict = {}


def prepare_common(We, be, gamma, beta, W0, b0, W1, b1, Wout, bout, use_gb):
    ident = np.eye(128, dtype=np.float32)
    selbc = np.zeros((2, 128), dtype=np.float32)
    selbc[0, 0:64] = 1.0
    selbc[1, 64:128] = 1.0

    def stack2(v):
        v = np.asarray(v, dtype=np.float32).reshape(-1)
        return np.concatenate([v, v]).reshape(128, 1)

    common = {
        "We": np.ascontiguousarray(np.asarray(We, dtype=np.float32)).reshape(
            2, 128, H).astype(np.float16),
        "W0s": np.concatenate([W0, W0]).astype(np.float16),
        "W1s": np.concatenate([W1, W1]).astype(np.float16),
        "Wouts": np.concatenate([Wout, Wout]).astype(np.float32),
        "be2": stack2(be), "b02": stack2(b0), "b12": stack2(b1),
        "bout": np.asarray(bout, dtype=np.float32).reshape(1, 1),
        "ident": ident.astype(np.float16),
        "selbcb": selbc.astype(np.float16),
        "allsel": (np.kron(np.eye(2, dtype=np.float32),
                           np.ones((64, 64), dtype=np.float32)) / 65536.0),
    }
    if use_gb:
        gT = np.ascontiguousarray(np.asarray(gamma, dtype=np.float32).T)
        bT = np.ascontiguousarray(np.asarray(beta, dtype=np.float32).T)
        common["gT2"] = np.concatenate([gT, gT]).astype(np.float32)
        common["bT2"] = np.concatenate([bT, bT]).astype(np.float32)
    return common


def prepare_core_map(common, x, mask, k, use_mask):
    xs = x[S * k:S * k + S]                       # [S, N, F_IN]
    xTs = np.ascontiguousarray(xs.transpose(0, 2, 1)).reshape(S, 2, 128, N)
    m = dict(common)
    m["xT"] = xTs.astype(np.float16)
    if use_mask:
        m["maskN"] = np.ascontiguousarray(mask[S * k:S * k + S])
    return m


def kernel(x, mask, We, be, gamma, beta, W0, b0, W1, b1, Wout, bout):
    x = np.ascontiguousarray(np.asarray(x, dtype=np.float32))
    mask = np.asarray(mask, dtype=np.float32)
    use_mask = bool(np.any(mask))
    use_gb = bool(np.any(np.asarray(gamma) != 1.0) or np.any(np.asarray(beta)))

    key = (use_mask, use_gb)
    if key not in _NC_CACHE:
        _NC_CACHE[key] = build_nc(use_mask, use_gb)
    nc = _NC_CACHE[key]

    common = prepare_common(We, be, gamma, beta, W0, b0, W1, b1, Wout, bout,
                            use_gb)
    in_maps = [prepare_core_map(common, x, mask, k, use_mask)
               for k in range(NCORES)]

    res = run_bass_kernel_spmd(nc, in_maps, list(range(NCORES)))
    global LAST_RESULT
    LAST_RESULT = res
    out = np.concatenate([res.results[k]["out"] for k in range(NCORES)], axis=0)
    return out.astype(np.float32)


LAST_RESULT = None
